# revision 1
# baseline (speedup 1.0000x reference)
"""Deformable Conv2D (nn_DeformableConv2D_81810537054370) Trainium2 Bass kernel.

Sharding: 8 cores = 4 batches x 2 groups (one (b, g) shard per core, zero
cross-core communication). Per core: offset conv (PE), bilinear index/weight
math (DVE), bilinear sampling via SWDGE dma_gather, combine (DVE), PE
transposes, folded depthwise+pointwise conv (PE).

Pixel permutation: within a 128-px image row, pixel px = 8*b + u lives on
gather-out partition pg = 16*u + b. This makes every idx-scatter DMA
expressible in <=3 dims with contiguous final dims. The permutation is
undone by the access patterns of the post-transpose copies.

Self-contained: hardcodes shapes; host prep is data-independent only.
"""

import sys

for _p in ("/opt/trn_rl_repo", "/root/.axon_site/_ro/trn_rl_repo"):
    if _p not in sys.path:
        sys.path.insert(0, _p)

import numpy as np
import ml_dtypes

import concourse.bass as bass
import concourse.mybir as mybir
import concourse.tile as tile
from concourse import bacc
from concourse.masks import make_identity

F32 = mybir.dt.float32
BF16 = mybir.dt.bfloat16
I16 = mybir.dt.int16
OP = mybir.AluOpType
AF = mybir.ActivationFunctionType

# problem constants
B, H, W, C = 4, 128, 128, 128
G = 2
Cg = C // G          # 64
K2 = 9
Kin = K2 * Cg        # 576
Fg = 64
WP = W + 1           # gather-table cols per image row (129)
NROW = H * WP        # 16512 gather rows
NPIX = H * W
STR = 16             # output rows per stripe
NSTRIPE = H // STR   # 8
WR = STR + 2         # sampled-row window per stripe (halo)
NI = 3 * WR * 128    # idxs per gather instr (3 taps) = 6912
SLOTS_I = NI // 16   # 432
SLOTS_S = 3 * SLOTS_I            # 1296 per stripe
SLOTS_T = NSTRIPE * SLOTS_S      # 10368 per corner
NCHUNK = 5           # 576 -> 5 chunks of 128 (last zero-padded)
NF = K2 * H          # 1152


def _build_program(debug=False):
    nc = bacc.Bacc("TRN2", target_bir_lowering=False, debug=False,
                   enable_asserts=False)
    dbg = {}
    with tile.TileContext(nc) as tc:
        with tc.tile_pool(name="dram", bufs=1, space="DRAM") as dram:
            xT_d = dram.tile([Cg, H + 2, W + 2], F32, kind="ExternalInput")
            xg_d = dram.tile([NROW, 2 * Cg], BF16, kind="ExternalInput")
            b0_d = dram.tile([2 * K2, NPIX], F32, kind="ExternalInput")
            offw_d = dram.tile([Cg, K2 * 2 * K2], F32, kind="ExternalInput")
            wd_d = dram.tile([128, K2 * NCHUNK * Fg], BF16, kind="ExternalInput")
            bfin_d = dram.tile([Fg, 1], F32, kind="ExternalInput")
            out_d = dram.tile([Fg, NPIX], F32, kind="ExternalOutput")
            locx_d = dram.tile([K2 * NPIX], F32)   # [k][px][py]
            locy_d = dram.tile([K2 * NPIX], F32)
            topT_d = dram.tile([K2, H, 128], I16)  # [k][py][px]
            botT_d = dram.tile([K2, H, 128], I16)
            if debug:
                dbg["loc"] = dram.tile([2 * K2, NPIX], F32, kind="ExternalOutput",
                                       name="dbg_loc")
                dbg["samp"] = dram.tile([128, NSTRIPE * WR * K2 * Cg], BF16,
                                        kind="ExternalOutput", name="dbg_samp")

            with tc.tile_pool(name="persist", bufs=1) as pp, \
                 tc.tile_pool(name="pidx", bufs=1) as pidx:
                topw = pidx.tile([128, SLOTS_T], I16)
                botw = pidx.tile([128, SLOTS_T], I16)
                nc.vector.memset(topw[:], 0)
                nc.vector.memset(botw[:], 0)
                wx0b = pp.tile([128, 1154], BF16)
                wx1b = pp.tile([128, 1154], BF16)
                wy0b = pp.tile([128, 1154], BF16)
                wy1b = pp.tile([128, 1154], BF16)
                wd_sb = pp.tile([128, K2 * NCHUNK * Fg], BF16)
                bfin = pp.tile([Fg, 1], F32)
                identb = pp.tile([128, 128], BF16)
                identf = pp.tile([128, 128], F32)

                nc.sync.dma_start(wd_sb[:], wd_d[:])
                nc.sync.dma_start(bfin[:], bfin_d[:])
                make_identity(nc, identb[:])
                make_identity(nc, identf[:])
                for wt in (wx0b, wx1b, wy0b, wy1b):
                    nc.vector.memset(wt[:, 0:1], 0.0)
                    nc.vector.memset(wt[:, 1153:1154], 0.0)

                # ---- phase 1: offset conv; locA free dim is px-major ----
                # ---- phase 2: bilinear math in pg-permuted partitions ----
                with tc.tile_pool(name="ph1", bufs=1) as p1x:
                    locA = p1x.tile([2 * K2, NPIX], F32)
                    with tc.tile_pool(name="ph1a", bufs=1) as p1a, \
                         tc.tile_pool(name="ph1b", bufs=2) as p1b, \
                         tc.tile_pool(name="ps1", bufs=2, space="PSUM") as ps1:
                      xT = p1a.tile([Cg, H + 2, W + 2], F32)
                      offw = p1a.tile([Cg, K2 * 2 * K2], F32)
                      nc.sync.dma_start(xT[:], xT_d[:])
                      nc.sync.dma_start(offw[:], offw_d[:])
                      for t in range(NPIX // 512):   # 4 px-columns per tile
                        c0 = t * 4
                        b0t = p1b.tile([2 * K2, 512], F32, tag="b0t")
                        nc.sync.dma_start(b0t[:], b0_d[:, t * 512:(t + 1) * 512])
                        pt = ps1.tile([2 * K2, 512], F32, space="PSUM")
                        for tap in range(K2):
                            dy, dx = tap // 3, tap % 3
                            rhs = xT[:, dy:dy + H,
                                     c0 + dx:c0 + dx + 4].rearrange(
                                         "c y x -> c x y")
                            nc.tensor.matmul(
                                out=pt[:],
                                lhsT=offw[:, tap * 18:(tap + 1) * 18],
                                rhs=rhs,
                                start=(tap == 0), stop=(tap == K2 - 1))
                        nc.vector.tensor_tensor(
                            out=locA[:, t * 512:(t + 1) * 512],
                            in0=pt[:], in1=b0t[:], op=OP.add)
                    if debug:
                        nc.sync.dma_start(dbg["loc"][:], locA[:])
                    # bounce locA through DRAM ([k][px][py])
                    nc.sync.dma_start(
                        locx_d[:].rearrange("(a b) -> a b", a=K2), locA[0:K2, :])
                    nc.sync.dma_start(
                        locy_d[:].rearrange("(a b) -> a b", a=K2),
                        locA[K2:2 * K2, :])

                with tc.tile_pool(name="ph2", bufs=1) as p1:
                    # phase 2 tiles (pg-partition order, free = (k, py))
                    locx = p1.tile([128, NF], F32)
                    locy = p1.tile([128, NF], F32)
                    nc.vector.memset(locx[:], 0.0)
                    nc.vector.memset(locy[:], 0.0)
                    # load in pg-partition order: partitions pg=16u+b,
                    # free (k, py); src px = 8b+u.
                    for (dst, src_d) in ((locx, locx_d), (locy, locy_d)):
                        sv = src_d[:].rearrange("(k x y) -> x k y", k=K2, x=W)
                        for u in range(8):
                            sap = sv[u::8]       # px = 8b+u, b=0..15
                            dd = dst[16 * u:16 * (u + 1), :].rearrange(
                                "p (k y) -> p k y", k=K2)
                            nc.sync.dma_start(dd, sap)

                    fr = p1.tile([128, NF], F32)
                    x0f = p1.tile([128, NF], F32)
                    x1f = p1.tile([128, NF], F32)
                    y0f = p1.tile([128, NF], F32)
                    y1f = p1.tile([128, NF], F32)
                    topf = p1.tile([128, NF], F32)
                    botf = p1.tile([128, NF], F32)

                    for loc, c0f, c1f, w0, w1 in (
                            (locx, x0f, x1f, wx0b, wx1b),
                            (locy, y0f, y1f, wy0b, wy1b)):
                        nc.vector.tensor_scalar(out=loc[:], in0=loc[:],
                                                scalar1=0.0, scalar2=float(W - 1),
                                                op0=OP.max, op1=OP.min)
                        # exact floor: r = round(loc) via 2^23 trick,
                        # then subtract 1 where r > loc
                        nc.vector.tensor_scalar(out=c0f[:], in0=loc[:],
                                                scalar1=8388608.0,
                                                scalar2=-8388608.0,
                                                op0=OP.add, op1=OP.add)
                        nc.vector.tensor_tensor(out=fr[:], in0=c0f[:],
                                                in1=loc[:], op=OP.is_gt)
                        nc.vector.tensor_sub(out=c0f[:], in0=c0f[:], in1=fr[:])
                        nc.vector.tensor_scalar(out=c1f[:], in0=c0f[:],
                                                scalar1=1.0, scalar2=float(W - 1),
                                                op0=OP.add, op1=OP.min)
                        nc.vector.tensor_sub(out=w0[:, 1:1153], in0=c1f[:],
                                             in1=loc[:])
                        nc.vector.tensor_sub(out=w1[:, 1:1153], in0=loc[:],
                                             in1=c0f[:])

                    nc.vector.scalar_tensor_tensor(
                        out=topf[:], in0=y0f[:], scalar=float(WP), in1=x0f[:],
                        op0=OP.mult, op1=OP.add)
                    nc.vector.scalar_tensor_tensor(
                        out=botf[:], in0=y1f[:], scalar=float(WP), in1=x0f[:],
                        op0=OP.mult, op1=OP.add)

                    # transpose each k-block to [py, px-natural] int16, then
                    # scatter into wrapped idx layout.
                    with tc.tile_pool(name="tpi", bufs=1) as tpi, \
                         tc.tile_pool(name="pst2", bufs=4, space="PSUM") as pst2:
                        for ci, (srcf, dsti) in enumerate(
                                ((topf, topw), (botf, botw))):
                            tT = [tpi.tile([128, 128], I16, name=f"tT{ci}_{k}")
                                  for k in range(K2)]
                            for k in range(K2):
                                ptr = pst2.tile([128, 128], F32, space="PSUM",
                                                tag="ptr")
                                nc.tensor.transpose(
                                    out=ptr[:],
                                    in_=srcf[:, k * H:(k + 1) * H],
                                    identity=identf[:])
                                # un-permute pg -> px while casting to int16
                                src = ptr[:].rearrange("p (u b) -> p u b", u=8)
                                dd = tT[k][:].rearrange("p (b u) -> p u b",
                                                        b=16)
                                nc.scalar.copy(out=dd, in_=src)
                            # bounce tT through DRAM [k][py][px], then
                            # scatter per (s, k) into the wrapped layout
                            tT_d = topT_d if dsti is topw else botT_d
                            for k in range(K2):
                                nc.sync.dma_start(tT_d[k, :, :], tT[k][:])
                            for k in range(K2):
                                g3, kl = k // 3, k % 3

                                def sc(s, w_lo, w_hi, py0, k=k, g3=g3, kl=kl):
                                    cnt = w_hi - w_lo
                                    src = tT_d[k, py0:py0 + cnt, :].rearrange(
                                        "w (b u) -> b w u", b=16)
                                    o0 = s * SLOTS_S + g3 * 432 + kl * 144 + \
                                        8 * w_lo
                                    dd = dsti[0:16, o0:o0 + cnt * 8].rearrange(
                                        "p (w u) -> p w u", u=8)
                                    nc.sync.dma_start(dd, src)

                                for s in range(NSTRIPE):
                                    if s == 0:
                                        sc(s, 0, 1, 0)
                                        sc(s, 1, WR, 0)
                                    elif s == NSTRIPE - 1:
                                        sc(s, 0, WR - 1, STR * s - 1)
                                        sc(s, WR - 1, WR, H - 1)
                                    else:
                                        sc(s, 0, WR, STR * s - 1)
                            for a in range(1, 8):
                                nc.sync.dma_start(dsti[16 * a:16 * (a + 1), :],
                                                  dsti[0:16, :])

                # ---- phase 3: gather / combine / transpose / dwpw ----
                with tc.tile_pool(name="gb", bufs=2) as gbp, \
                     tc.tile_pool(name="cmb", bufs=2) as cmb, \
                     tc.tile_pool(name="samp", bufs=1) as smp, \
                     tc.tile_pool(name="outp", bufs=2) as outp, \
                     tc.tile_pool(name="pst", bufs=4, space="PSUM") as pst, \
                     tc.tile_pool(name="psm", bufs=2, space="PSUM") as psm:
                    samp = smp.tile([128, WR, K2, Cg], BF16)
                    sampT = [smp.tile([128, WR, W + 2], BF16, name=f"sampT{i}")
                             for i in range(NCHUNK)]
                    for i in range(NCHUNK):
                        nc.vector.memset(sampT[i][:, :, 0:1], 0.0)
                        nc.vector.memset(sampT[i][:, :, W + 1:W + 2], 0.0)
                    nc.vector.memset(sampT[4][64:128, :, :], 0.0)

                    import os as _os
                    _ns = int(_os.environ.get("KSTRIPES", NSTRIPE))
                    for s in range(_ns):
                        for k in range(K2):
                            g3, kl = k // 3, k % 3
                            gbt = gbp.tile([128, WR, 2 * Cg], BF16, tag="gt")
                            gbb = gbp.tile([128, WR, 2 * Cg], BF16, tag="gb")
                            off = s * SLOTS_S + g3 * SLOTS_I + kl * 144
                            for gout, wtens in ((gbt, topw), (gbb, botw)):
                                for c3 in range(3):   # 6 w-rows per gather
                                    nc.gpsimd.dma_gather(
                                        out_ap=gout[:, 6 * c3:6 * (c3 + 1), :],
                                        in_ap=xg_d[:],
                                        idxs_ap=wtens[:, off + 48 * c3:
                                                      off + 48 * (c3 + 1)],
                                        num_idxs=768, num_idxs_reg=768,
                                        elem_size=2 * Cg)
                            if int(_os.environ.get("KPARTS", 4)) < 2:
                                continue
                            if True:
                                a_ = gbt[:, :, 0:Cg]
                                c_ = gbt[:, :, Cg:2 * Cg]
                                b_ = gbb[:, :, 0:Cg]
                                d_ = gbb[:, :, Cg:2 * Cg]
                                wsl = slice(k * H + STR * s, k * H + STR * s + WR)
                                wx0 = wx0b[:, wsl].to_broadcast([128, WR, Cg])
                                wx1 = wx1b[:, wsl].to_broadcast([128, WR, Cg])
                                wy0 = wy0b[:, wsl].to_broadcast([128, WR, Cg])
                                wy1 = wy1b[:, wsl].to_broadcast([128, WR, Cg])
                                t1 = cmb.tile([128, WR, Cg], BF16, tag="t1")
                                t2 = cmb.tile([128, WR, Cg], BF16, tag="t2")
                                t3 = cmb.tile([128, WR, Cg], BF16, tag="t3")
                                nc.vector.tensor_tensor(out=t1[:], in0=a_, in1=wx0, op=OP.mult)
                                nc.vector.tensor_tensor(out=t2[:], in0=c_, in1=wx1, op=OP.mult)
                                nc.vector.tensor_tensor(out=t1[:], in0=t1[:], in1=t2[:], op=OP.add)
                                nc.vector.tensor_tensor(out=t2[:], in0=b_, in1=wx0, op=OP.mult)
                                nc.vector.tensor_tensor(out=t3[:], in0=d_, in1=wx1, op=OP.mult)
                                nc.vector.tensor_tensor(out=t2[:], in0=t2[:], in1=t3[:], op=OP.add)
                                nc.vector.tensor_tensor(out=t1[:], in0=t1[:], in1=wy0, op=OP.mult)
                                nc.vector.tensor_tensor(out=t2[:], in0=t2[:], in1=wy1, op=OP.mult)
                                nc.vector.tensor_tensor(
                                    out=samp[:, :, k, :], in0=t1[:], in1=t2[:], op=OP.add)
                        if debug:
                            nc.sync.dma_start(
                                dbg["samp"][:, s * WR * Kin:(s + 1) * WR * Kin],
                                samp[:].rearrange("p a b c -> p (a b c)"))
                        if int(_os.environ.get("KPARTS", 4)) < 3:
                            continue
                        # transposes into sampT (un-permuting pg -> px)
                        w_lo = 1 if s == 0 else 0
                        w_hi = WR - 1 if s == NSTRIPE - 1 else WR
                        if s == 0:
                            for i in range(NCHUNK):
                                nc.vector.memset(sampT[i][:, 0, :], 0.0)
                        if s == NSTRIPE - 1:
                            for i in range(NCHUNK):
                                nc.vector.memset(sampT[i][:, WR - 1, :], 0.0)
                        for wrow in range(w_lo, w_hi):
                            for kp in range(NCHUNK):
                                kk = 2 * kp
                                width = 128 if kp < 4 else 64
                                src = samp[:, wrow, kk:kk + (2 if kp < 4 else 1), :]
                                ptt = pst.tile([128, 128], BF16, space="PSUM",
                                               tag="ptt")
                                nc.tensor.transpose(
                                    out=ptt[:width, :],
                                    in_=src.rearrange("p a b -> p (a b)"),
                                    identity=identb[:])
                                src2 = ptt[:width, :].rearrange(
                                    "p (u b) -> p u b", u=8)
                                dd = sampT[kp][:width, wrow, 1:1 + W].rearrange(
                                    "p (b u) -> p u b", b=16)
                                nc.scalar.copy(out=dd, in_=src2)
                        if int(_os.environ.get("KPARTS", 4)) < 4:
                            continue
                        # dwpw matmuls
                        for t in range(4):
                            pm = psm.tile([Fg, 512], F32, space="PSUM", tag="pm")
                            first = True
                            for dy in (-1, 0, 1):
                                for dx in (-1, 0, 1):
                                    d_i = (dy + 1) * 3 + (dx + 1)
                                    for ci in range(NCHUNK):
                                        lhs = wd_sb[:, (d_i * NCHUNK + ci) * Fg:
                                                    (d_i * NCHUNK + ci + 1) * Fg]
                                        wr0 = t * 4 + 1 + dy
                                        rhs = sampT[ci][:, wr0:wr0 + 4,
                                                        1 + dx:1 + dx + W]
                                        last = (dy == 1 and dx == 1 and
                                                ci == NCHUNK - 1)
                                        nc.tensor.matmul(out=pm[:], lhsT=lhs,
                                                         rhs=rhs, start=first,
                                                         stop=last)
                                        first = False
                            ot = outp.tile([Fg, 512], F32, tag="ot")
                            nc.scalar.activation(out=ot[:], in_=pm[:],
                                                 func=AF.Identity, bias=bfin[:],
                                                 scale=1.0)
                            nc.sync.dma_start(
                                out_d[:, s * 2048 + t * 512:
                                      s * 2048 + (t + 1) * 512],
                                ot[:])
    nc.compile()
    names = dict(xT=xT_d.name, xg=xg_d.name, b0=b0_d.name, offw=offw_d.name,
                 wd=wd_d.name, bfin=bfin_d.name, out=out_d.name,
                 dbg={k: v.name for k, v in dbg.items()})
    return nc, names


def _host_prep(x, off_w, off_b, dw_w, dw_b, pw_w, pw_b, b, g):
    """Data-independent prep of one (b, g) shard's device inputs."""
    xi = np.asarray(x)[b, :, :, g * Cg:(g + 1) * Cg].astype(np.float32)
    xT = np.zeros((Cg, H + 2, W + 2), np.float32)
    xT[:, 1:H + 1, 1:W + 1] = xi.transpose(2, 0, 1)
    # gather table rows (y, x'): [x(y, min(x',127)), x(y, min(x'+1,127))]
    xgl = np.pad(xi, ((0, 0), (0, 2), (0, 0)), mode="edge")
    xg = np.concatenate([xgl[:, :WP], xgl[:, 1:WP + 1]], axis=2)
    xg = xg.reshape(NROW, 2 * Cg).astype(ml_dtypes.bfloat16)
    # base tables [18, NPIX], free dim px-major (px*H + py)
    lin = np.array([-1.0, 0.0, 1.0], np.float32)
    gx, gy = np.meshgrid(np.arange(W, dtype=np.float32),
                         np.arange(H, dtype=np.float32))
    gxT, gyT = gx.T.reshape(-1), gy.T.reshape(-1)   # px-major flatten
    ob = np.asarray(off_b)[g].astype(np.float32)
    b0 = np.zeros((2 * K2, NPIX), np.float32)
    for k in range(K2):
        b0[k] = gxT + lin[k % 3] + ob[2 * k]
        b0[K2 + k] = gyT + lin[k // 3] + ob[2 * k + 1]
    ow = np.asarray(off_w)[g].astype(np.float32)
    offw = np.zeros((Cg, K2 * 2 * K2), np.float32)
    for tap in range(K2):
        wt = ow[tap // 3, tap % 3]
        offw[:, tap * 18:tap * 18 + K2] = wt[:, 0::2]
        offw[:, tap * 18 + K2:(tap + 1) * 18] = wt[:, 1::2]
    dw = np.asarray(dw_w)[g, :, :, 0, :].astype(np.float32)
    pw = np.asarray(pw_w)[g, 0, 0].astype(np.float32)
    wd = np.zeros((128, K2 * NCHUNK * Fg), np.float32)
    for d_i in range(K2):
        wfull = dw[d_i // 3, d_i % 3][:, None] * pw
        for ci in range(NCHUNK):
            rows = min(128, Kin - ci * 128)
            wd[:rows, (d_i * NCHUNK + ci) * Fg:(d_i * NCHUNK + ci + 1) * Fg] = \
                wfull[ci * 128:ci * 128 + rows]
    wd = wd.astype(ml_dtypes.bfloat16)
    bfin = (pw.T @ np.asarray(dw_b)[g].astype(np.float32)
            + np.asarray(pw_b)[g].astype(np.float32)).reshape(Fg, 1)
    return dict(xT=xT, xg=xg, b0=b0, offw=offw, wd=wd, bfin=bfin)


_CACHE = {}


def _get_program(debug=False):
    key = ("prog", debug)
    if key not in _CACHE:
        _CACHE[key] = _build_program(debug=debug)
    return _CACHE[key]


def kernel(x, off_w, off_b, dw_w, dw_b, pw_w, pw_b):
    from concourse import bass_utils
    nc, names = _get_program()
    shards = [(b, g) for b in range(B) for g in range(G)]
    in_maps = []
    for b, g in shards:
        prep = _host_prep(x, off_w, off_b, dw_w, dw_b, pw_w, pw_b, b, g)
        in_maps.append({names[k]: v for k, v in prep.items()})
    res = bass_utils.run_bass_kernel_spmd(nc, in_maps, core_ids=list(range(8)))
    out = np.zeros((B, H, W, C), np.float32)
    for i, (b, g) in enumerate(shards):
        o = np.asarray(res.results[i][names["out"]])  # [Fg, NPIX]
        out[b, :, :, g * Cg:(g + 1) * Cg] = \
            o.reshape(Fg, H, W).transpose(1, 2, 0)
    return out



# revision 2
# speedup vs baseline: 822.3963x; 822.3963x over previous
"""Deformable Conv2D (nn_DeformableConv2D_81810537054370) Trainium2 Bass kernel.

Sharding: 8 cores = 4 batches x 2 groups (one (b, g) shard per core, zero
cross-core communication). Per core: offset conv (PE), bilinear index/weight
math (DVE), bilinear sampling via SWDGE dma_gather, combine (DVE), PE
transposes, folded depthwise+pointwise conv (PE).

Pixel permutation: within a 128-px image row, pixel px = 8*b + u lives on
gather-out partition pg = 16*u + b. This makes every idx-scatter DMA
expressible in <=3 dims with contiguous final dims. The permutation is
undone by the access patterns of the post-transpose copies.

Self-contained: hardcodes shapes; host prep is data-independent only.
"""

import sys

for _p in ("/opt/trn_rl_repo", "/root/.axon_site/_ro/trn_rl_repo"):
    if _p not in sys.path:
        sys.path.insert(0, _p)

import numpy as np
import ml_dtypes

import concourse.bass as bass
import concourse.mybir as mybir
import concourse.tile as tile
from concourse import bacc
from concourse.masks import make_identity

F32 = mybir.dt.float32
BF16 = mybir.dt.bfloat16
I16 = mybir.dt.int16
OP = mybir.AluOpType
AF = mybir.ActivationFunctionType

# problem constants
B, H, W, C = 4, 128, 128, 128
G = 2
Cg = C // G          # 64
K2 = 9
Kin = K2 * Cg        # 576
Fg = 64
WP = W + 1           # gather-table cols per image row (129)
NROW = H * WP        # 16512 gather rows
NPIX = H * W
STR = 16             # output rows per stripe
NSTRIPE = H // STR   # 8
WR = STR + 2         # sampled-row window per stripe (halo)
NI = 3 * WR * 128    # idxs per gather instr (3 taps) = 6912
SLOTS_I = NI // 16   # 432
SLOTS_S = 3 * SLOTS_I            # 1296 per stripe
SLOTS_T = NSTRIPE * SLOTS_S      # 10368 per corner
NCHUNK = 5           # 576 -> 5 chunks of 128 (last zero-padded)
NF = K2 * H          # 1152


def _build_program(debug=False):
    nc = bacc.Bacc("TRN2", target_bir_lowering=False, debug=False,
                   enable_asserts=False)
    dbg = {}
    with tile.TileContext(nc) as tc:
        with tc.tile_pool(name="dram", bufs=1, space="DRAM") as dram:
            xT_d = dram.tile([Cg, H + 2, W + 2], F32, kind="ExternalInput")
            xg_d = dram.tile([NROW, 2 * Cg], BF16, kind="ExternalInput")
            b0_d = dram.tile([2 * K2, NPIX], F32, kind="ExternalInput")
            offw_d = dram.tile([Cg, K2 * 2 * K2], F32, kind="ExternalInput")
            wd_d = dram.tile([128, K2 * NCHUNK * Fg], BF16, kind="ExternalInput")
            bfin_d = dram.tile([Fg, 1], F32, kind="ExternalInput")
            out_d = dram.tile([Fg, NPIX], F32, kind="ExternalOutput")
            locx_d = dram.tile([K2 * NPIX], F32)   # [k][px][py]
            locy_d = dram.tile([K2 * NPIX], F32)
            topT_d = dram.tile([K2, H, 128], I16)  # [k][py][px]
            botT_d = dram.tile([K2, H, 128], I16)
            if debug:
                dbg["loc"] = dram.tile([2 * K2, NPIX], F32, kind="ExternalOutput",
                                       name="dbg_loc")
                dbg["samp"] = dram.tile([128, NSTRIPE * WR * K2 * Cg], BF16,
                                        kind="ExternalOutput", name="dbg_samp")

            with tc.tile_pool(name="persist", bufs=1) as pp, \
                 tc.tile_pool(name="pidx", bufs=1) as pidx:
                topw = pidx.tile([128, SLOTS_T], I16)
                botw = pidx.tile([128, SLOTS_T], I16)
                nc.vector.memset(topw[:], 0)
                nc.vector.memset(botw[:], 0)
                wx0b = pp.tile([128, 1154], BF16)
                wx1b = pp.tile([128, 1154], BF16)
                wy0b = pp.tile([128, 1154], BF16)
                wy1b = pp.tile([128, 1154], BF16)
                wd_sb = pp.tile([128, K2 * NCHUNK * Fg], BF16)
                bfin = pp.tile([Fg, 1], F32)
                identb = pp.tile([128, 128], BF16)
                identf = pp.tile([128, 128], F32)

                nc.sync.dma_start(wd_sb[:], wd_d[:])
                nc.sync.dma_start(bfin[:], bfin_d[:])
                make_identity(nc, identb[:])
                make_identity(nc, identf[:])
                for wt in (wx0b, wx1b, wy0b, wy1b):
                    nc.vector.memset(wt[:, 0:1], 0.0)
                    nc.vector.memset(wt[:, 1153:1154], 0.0)

                # ---- phase 1: offset conv; locA free dim is px-major ----
                # ---- phase 2: bilinear math in pg-permuted partitions ----
                with tc.tile_pool(name="ph1", bufs=1) as p1x:
                    locA = p1x.tile([2 * K2, NPIX], F32)
                    with tc.tile_pool(name="ph1a", bufs=1) as p1a, \
                         tc.tile_pool(name="ph1b", bufs=2) as p1b, \
                         tc.tile_pool(name="ps1", bufs=2, space="PSUM") as ps1:
                      xT = p1a.tile([Cg, H + 2, W + 2], F32)
                      offw = p1a.tile([Cg, K2 * 2 * K2], F32)
                      nc.sync.dma_start(xT[:], xT_d[:])
                      nc.sync.dma_start(offw[:], offw_d[:])
                      for t in range(NPIX // 512):   # 4 px-columns per tile
                        c0 = t * 4
                        b0t = p1b.tile([2 * K2, 512], F32, tag="b0t")
                        nc.sync.dma_start(b0t[:], b0_d[:, t * 512:(t + 1) * 512])
                        pt = ps1.tile([2 * K2, 512], F32, space="PSUM")
                        for tap in range(K2):
                            dy, dx = tap // 3, tap % 3
                            rhs = xT[:, dy:dy + H,
                                     c0 + dx:c0 + dx + 4].rearrange(
                                         "c y x -> c x y")
                            nc.tensor.matmul(
                                out=pt[:],
                                lhsT=offw[:, tap * 18:(tap + 1) * 18],
                                rhs=rhs,
                                start=(tap == 0), stop=(tap == K2 - 1))
                        nc.vector.tensor_tensor(
                            out=locA[:, t * 512:(t + 1) * 512],
                            in0=pt[:], in1=b0t[:], op=OP.add)
                    if debug:
                        nc.sync.dma_start(dbg["loc"][:], locA[:])
                    # bounce locA through DRAM ([k][px][py])
                    nc.sync.dma_start(
                        locx_d[:].rearrange("(a b) -> a b", a=K2), locA[0:K2, :])
                    nc.sync.dma_start(
                        locy_d[:].rearrange("(a b) -> a b", a=K2),
                        locA[K2:2 * K2, :])

                with tc.tile_pool(name="ph2", bufs=1) as p1:
                    # phase 2 tiles (pg-partition order, free = (k, py))
                    locx = p1.tile([128, NF], F32)
                    locy = p1.tile([128, NF], F32)
                    nc.vector.memset(locx[:], 0.0)
                    nc.vector.memset(locy[:], 0.0)
                    # load in pg-partition order: partitions pg=16u+b,
                    # free (k, py); src px = 8b+u.
                    for (dst, src_d) in ((locx, locx_d), (locy, locy_d)):
                        sv = src_d[:].rearrange("(k x y) -> x k y", k=K2, x=W)
                        for u in range(8):
                            sap = sv[u::8]       # px = 8b+u, b=0..15
                            dd = dst[16 * u:16 * (u + 1), :].rearrange(
                                "p (k y) -> p k y", k=K2)
                            nc.sync.dma_start(dd, sap)

                    fr = p1.tile([128, NF], F32)
                    x0f = p1.tile([128, NF], F32)
                    x1f = p1.tile([128, NF], F32)
                    y0f = p1.tile([128, NF], F32)
                    y1f = p1.tile([128, NF], F32)
                    topf = p1.tile([128, NF], F32)
                    botf = p1.tile([128, NF], F32)

                    for loc, c0f, c1f, w0, w1 in (
                            (locx, x0f, x1f, wx0b, wx1b),
                            (locy, y0f, y1f, wy0b, wy1b)):
                        nc.vector.tensor_scalar(out=loc[:], in0=loc[:],
                                                scalar1=0.0, scalar2=float(W - 1),
                                                op0=OP.max, op1=OP.min)
                        # exact floor: r = round(loc) via 2^23 trick,
                        # then subtract 1 where r > loc
                        nc.vector.tensor_scalar(out=c0f[:], in0=loc[:],
                                                scalar1=8388608.0,
                                                scalar2=-8388608.0,
                                                op0=OP.add, op1=OP.add)
                        nc.vector.tensor_tensor(out=fr[:], in0=c0f[:],
                                                in1=loc[:], op=OP.is_gt)
                        nc.vector.tensor_sub(out=c0f[:], in0=c0f[:], in1=fr[:])
                        nc.vector.tensor_scalar(out=c1f[:], in0=c0f[:],
                                                scalar1=1.0, scalar2=float(W - 1),
                                                op0=OP.add, op1=OP.min)
                        nc.vector.tensor_sub(out=w0[:, 1:1153], in0=c1f[:],
                                             in1=loc[:])
                        nc.vector.tensor_sub(out=w1[:, 1:1153], in0=loc[:],
                                             in1=c0f[:])

                    nc.vector.scalar_tensor_tensor(
                        out=topf[:], in0=y0f[:], scalar=float(WP), in1=x0f[:],
                        op0=OP.mult, op1=OP.add)
                    nc.vector.scalar_tensor_tensor(
                        out=botf[:], in0=y1f[:], scalar=float(WP), in1=x0f[:],
                        op0=OP.mult, op1=OP.add)

                    # transpose each k-block to [py, px-natural] int16, then
                    # scatter into wrapped idx layout.
                    with tc.tile_pool(name="tpi", bufs=1) as tpi, \
                         tc.tile_pool(name="pst2", bufs=4, space="PSUM") as pst2:
                        for ci, (srcf, dsti) in enumerate(
                                ((topf, topw), (botf, botw))):
                            tT = [tpi.tile([128, 128], I16, name=f"tT{ci}_{k}")
                                  for k in range(K2)]
                            for k in range(K2):
                                ptr = pst2.tile([128, 128], F32, space="PSUM",
                                                tag="ptr")
                                nc.tensor.transpose(
                                    out=ptr[:],
                                    in_=srcf[:, k * H:(k + 1) * H],
                                    identity=identf[:])
                                # un-permute pg -> px while casting to int16
                                src = ptr[:].rearrange("p (u b) -> p u b", u=8)
                                dd = tT[k][:].rearrange("p (b u) -> p u b",
                                                        b=16)
                                nc.scalar.copy(out=dd, in_=src)
                            # bounce tT through DRAM [k][py][px], then
                            # scatter per (s, k) into the wrapped layout
                            tT_d = topT_d if dsti is topw else botT_d
                            for k in range(K2):
                                nc.sync.dma_start(tT_d[k, :, :], tT[k][:])
                            for k in range(K2):
                                g3, kl = k // 3, k % 3

                                def sc(s, w_lo, w_hi, py0, k=k, g3=g3, kl=kl):
                                    cnt = w_hi - w_lo
                                    src = tT_d[k, py0:py0 + cnt, :].rearrange(
                                        "w (b u) -> b w u", b=16)
                                    o0 = s * SLOTS_S + g3 * 432 + kl * 144 + \
                                        8 * w_lo
                                    dd = dsti[0:16, o0:o0 + cnt * 8].rearrange(
                                        "p (w u) -> p w u", u=8)
                                    nc.sync.dma_start(dd, src)

                                for s in range(NSTRIPE):
                                    if s == 0:
                                        sc(s, 0, 1, 0)
                                        sc(s, 1, WR, 0)
                                    elif s == NSTRIPE - 1:
                                        sc(s, 0, WR - 1, STR * s - 1)
                                        sc(s, WR - 1, WR, H - 1)
                                    else:
                                        sc(s, 0, WR, STR * s - 1)
                            for a in range(1, 8):
                                nc.sync.dma_start(dsti[16 * a:16 * (a + 1), :],
                                                  dsti[0:16, :])

                # ---- phase 3: gather / combine / transpose / dwpw ----
                with tc.tile_pool(name="gb", bufs=2) as gbp, \
                     tc.tile_pool(name="cmb", bufs=2) as cmb, \
                     tc.tile_pool(name="samp", bufs=1) as smp, \
                     tc.tile_pool(name="outp", bufs=2) as outp, \
                     tc.tile_pool(name="pst", bufs=4, space="PSUM") as pst, \
                     tc.tile_pool(name="psm", bufs=2, space="PSUM") as psm:
                    samp = smp.tile([128, WR, K2, Cg], BF16)
                    sampT = [smp.tile([128, WR, W + 2], BF16, name=f"sampT{i}")
                             for i in range(NCHUNK)]
                    for i in range(NCHUNK):
                        nc.vector.memset(sampT[i][:, :, 0:1], 0.0)
                        nc.vector.memset(sampT[i][:, :, W + 1:W + 2], 0.0)
                    nc.vector.memset(sampT[4][64:128, :, :], 0.0)

                    import os as _os
                    _ns = int(_os.environ.get("KSTRIPES", NSTRIPE))
                    for s in range(_ns):
                        for k in range(K2):
                            g3, kl = k // 3, k % 3
                            gbt = gbp.tile([128, WR, 2 * Cg], BF16, tag="gt")
                            gbb = gbp.tile([128, WR, 2 * Cg], BF16, tag="gb")
                            off = s * SLOTS_S + g3 * SLOTS_I + kl * 144
                            for gout, wtens in ((gbt, topw), (gbb, botw)):
                                for c3 in range(3):   # 6 w-rows per gather
                                    nc.gpsimd.dma_gather(
                                        out_ap=gout[:, 6 * c3:6 * (c3 + 1), :],
                                        in_ap=xg_d[:],
                                        idxs_ap=wtens[:, off + 48 * c3:
                                                      off + 48 * (c3 + 1)],
                                        num_idxs=768, num_idxs_reg=768,
                                        elem_size=2 * Cg)
                            if int(_os.environ.get("KPARTS", 4)) < 2:
                                continue
                            if True:
                                a_ = gbt[:, :, 0:Cg]
                                c_ = gbt[:, :, Cg:2 * Cg]
                                b_ = gbb[:, :, 0:Cg]
                                d_ = gbb[:, :, Cg:2 * Cg]
                                wsl = slice(k * H + STR * s, k * H + STR * s + WR)
                                wx0 = wx0b[:, wsl].to_broadcast([128, WR, Cg])
                                wx1 = wx1b[:, wsl].to_broadcast([128, WR, Cg])
                                wy0 = wy0b[:, wsl].to_broadcast([128, WR, Cg])
                                wy1 = wy1b[:, wsl].to_broadcast([128, WR, Cg])
                                t1 = cmb.tile([128, WR, Cg], BF16, tag="t1")
                                t2 = cmb.tile([128, WR, Cg], BF16, tag="t2")
                                t3 = cmb.tile([128, WR, Cg], BF16, tag="t3")
                                nc.vector.tensor_tensor(out=t1[:], in0=a_, in1=wx0, op=OP.mult)
                                nc.vector.tensor_tensor(out=t2[:], in0=c_, in1=wx1, op=OP.mult)
                                nc.vector.tensor_tensor(out=t1[:], in0=t1[:], in1=t2[:], op=OP.add)
                                nc.vector.tensor_tensor(out=t2[:], in0=b_, in1=wx0, op=OP.mult)
                                nc.vector.tensor_tensor(out=t3[:], in0=d_, in1=wx1, op=OP.mult)
                                nc.vector.tensor_tensor(out=t2[:], in0=t2[:], in1=t3[:], op=OP.add)
                                nc.vector.tensor_tensor(out=t1[:], in0=t1[:], in1=wy0, op=OP.mult)
                                nc.vector.tensor_tensor(out=t2[:], in0=t2[:], in1=wy1, op=OP.mult)
                                nc.vector.tensor_tensor(
                                    out=samp[:, :, k, :], in0=t1[:], in1=t2[:], op=OP.add)
                        if debug:
                            nc.sync.dma_start(
                                dbg["samp"][:, s * WR * Kin:(s + 1) * WR * Kin],
                                samp[:].rearrange("p a b c -> p (a b c)"))
                        if int(_os.environ.get("KPARTS", 4)) < 3:
                            continue
                        # transposes into sampT (un-permuting pg -> px)
                        w_lo = 1 if s == 0 else 0
                        w_hi = WR - 1 if s == NSTRIPE - 1 else WR
                        if s == 0:
                            for i in range(NCHUNK):
                                nc.vector.memset(sampT[i][:, 0, :], 0.0)
                        if s == NSTRIPE - 1:
                            for i in range(NCHUNK):
                                nc.vector.memset(sampT[i][:, WR - 1, :], 0.0)
                        for wrow in range(w_lo, w_hi):
                            for kp in range(NCHUNK):
                                kk = 2 * kp
                                width = 128 if kp < 4 else 64
                                src = samp[:, wrow, kk:kk + (2 if kp < 4 else 1), :]
                                ptt = pst.tile([128, 128], BF16, space="PSUM",
                                               tag="ptt")
                                nc.tensor.transpose(
                                    out=ptt[:width, :],
                                    in_=src.rearrange("p a b -> p (a b)"),
                                    identity=identb[:])
                                src2 = ptt[:width, :].rearrange(
                                    "p (u b) -> p u b", u=8)
                                dd = sampT[kp][:width, wrow, 1:1 + W].rearrange(
                                    "p (b u) -> p u b", b=16)
                                nc.scalar.copy(out=dd, in_=src2)
                        if int(_os.environ.get("KPARTS", 4)) < 4:
                            continue
                        # dwpw matmuls
                        for t in range(4):
                            pm = psm.tile([Fg, 512], F32, space="PSUM", tag="pm")
                            first = True
                            for dy in (-1, 0, 1):
                                for dx in (-1, 0, 1):
                                    d_i = (dy + 1) * 3 + (dx + 1)
                                    for ci in range(NCHUNK):
                                        lhs = wd_sb[:, (d_i * NCHUNK + ci) * Fg:
                                                    (d_i * NCHUNK + ci + 1) * Fg]
                                        wr0 = t * 4 + 1 + dy
                                        rhs = sampT[ci][:, wr0:wr0 + 4,
                                                        1 + dx:1 + dx + W]
                                        last = (dy == 1 and dx == 1 and
                                                ci == NCHUNK - 1)
                                        nc.tensor.matmul(out=pm[:], lhsT=lhs,
                                                         rhs=rhs, start=first,
                                                         stop=last)
                                        first = False
                            ot = outp.tile([Fg, 512], F32, tag="ot")
                            nc.scalar.activation(out=ot[:], in_=pm[:],
                                                 func=AF.Identity, bias=bfin[:],
                                                 scale=1.0)
                            nc.sync.dma_start(
                                out_d[:, s * 2048 + t * 512:
                                      s * 2048 + (t + 1) * 512],
                                ot[:])
    nc.compile()
    names = dict(xT=xT_d.name, xg=xg_d.name, b0=b0_d.name, offw=offw_d.name,
                 wd=wd_d.name, bfin=bfin_d.name, out=out_d.name,
                 dbg={k: v.name for k, v in dbg.items()})
    return nc, names


def _host_prep(x, off_w, off_b, dw_w, dw_b, pw_w, pw_b, b, g):
    """Data-independent prep of one (b, g) shard's device inputs."""
    xi = np.asarray(x)[b, :, :, g * Cg:(g + 1) * Cg].astype(np.float32)
    xT = np.zeros((Cg, H + 2, W + 2), np.float32)
    xT[:, 1:H + 1, 1:W + 1] = xi.transpose(2, 0, 1)
    # gather table rows (y, x'): [x(y, min(x',127)), x(y, min(x'+1,127))]
    xgl = np.pad(xi, ((0, 0), (0, 2), (0, 0)), mode="edge")
    xg = np.concatenate([xgl[:, :WP], xgl[:, 1:WP + 1]], axis=2)
    xg = xg.reshape(NROW, 2 * Cg).astype(ml_dtypes.bfloat16)
    # base tables [18, NPIX], free dim px-major (px*H + py)
    lin = np.array([-1.0, 0.0, 1.0], np.float32)
    gx, gy = np.meshgrid(np.arange(W, dtype=np.float32),
                         np.arange(H, dtype=np.float32))
    gxT, gyT = gx.T.reshape(-1), gy.T.reshape(-1)   # px-major flatten
    ob = np.asarray(off_b)[g].astype(np.float32)
    b0 = np.zeros((2 * K2, NPIX), np.float32)
    for k in range(K2):
        b0[k] = gxT + lin[k % 3] + ob[2 * k]
        b0[K2 + k] = gyT + lin[k // 3] + ob[2 * k + 1]
    ow = np.asarray(off_w)[g].astype(np.float32)
    offw = np.zeros((Cg, K2 * 2 * K2), np.float32)
    for tap in range(K2):
        wt = ow[tap // 3, tap % 3]
        offw[:, tap * 18:tap * 18 + K2] = wt[:, 0::2]
        offw[:, tap * 18 + K2:(tap + 1) * 18] = wt[:, 1::2]
    dw = np.asarray(dw_w)[g, :, :, 0, :].astype(np.float32)
    pw = np.asarray(pw_w)[g, 0, 0].astype(np.float32)
    wd = np.zeros((128, K2 * NCHUNK * Fg), np.float32)
    for d_i in range(K2):
        wfull = dw[d_i // 3, d_i % 3][:, None] * pw
        for ci in range(NCHUNK):
            rows = min(128, Kin - ci * 128)
            wd[:rows, (d_i * NCHUNK + ci) * Fg:(d_i * NCHUNK + ci + 1) * Fg] = \
                wfull[ci * 128:ci * 128 + rows]
    wd = wd.astype(ml_dtypes.bfloat16)
    bfin = (pw.T @ np.asarray(dw_b)[g].astype(np.float32)
            + np.asarray(pw_b)[g].astype(np.float32)).reshape(Fg, 1)
    return dict(xT=xT, xg=xg, b0=b0, offw=offw, wd=wd, bfin=bfin)


_CACHE = {}


def _get_program(debug=False):
    key = ("prog", debug)
    if key not in _CACHE:
        _CACHE[key] = _build_program(debug=debug)
    return _CACHE[key]


LAST_RESULT = None


def kernel(x, off_w, off_b, dw_w, dw_b, pw_w, pw_b):
    import os
    from concourse import bass_utils
    nc, names = _get_program()
    shards = [(b, g) for b in range(B) for g in range(G)]
    in_maps = []
    for b, g in shards:
        prep = _host_prep(x, off_w, off_b, dw_w, dw_b, pw_w, pw_b, b, g)
        in_maps.append({names[k]: v for k, v in prep.items()})
    kw = {}
    if os.environ.get("KERNEL_TRACE") == "1":
        kw = dict(trace=True)
        td = os.environ.get("KERNEL_TRACE_DIR")
        if td:
            kw["tmpdir"] = td
    res = bass_utils.run_bass_kernel_spmd(nc, in_maps, core_ids=list(range(8)),
                                          **kw)
    global LAST_RESULT
    LAST_RESULT = res
    out = np.zeros((B, H, W, C), np.float32)
    for i, (b, g) in enumerate(shards):
        o = np.asarray(res.results[i][names["out"]])  # [Fg, NPIX]
        out[b, :, :, g * Cg:(g + 1) * Cg] = \
            o.reshape(Fg, H, W).transpose(1, 2, 0)
    return out



# revision 7
# speedup vs baseline: 1236.9865x; 1.5041x over previous
"""Deformable Conv2D (nn_DeformableConv2D_81810537054370) Trainium2 Bass kernel.

Sharding: 8 cores = 4 batches x 2 groups (one (b, g) shard per core, zero
cross-core communication). Per core: offset conv (PE, bf16), bilinear
index/weight math (DVE), bilinear sampling via SWDGE dma_gather (one 512B
descriptor per (pixel, tap) carrying the whole 2x2 corner block), combine
(DVE, pre-multiplied corner weights), PE transposes, folded
depthwise+pointwise conv (PE).

Pixel permutation: within a 128-px image row, pixel px = 8*b + u lives on
gather-out partition pg = 16*u + b. This makes every idx-scatter DMA
expressible in <=3 dims with contiguous final dims. The permutation is
undone by the access patterns of the post-transpose copies.

Gather table: row r = x0*129 + y0 holds [X[y0,x0,:], X[y0,x0+1,:]] and the
next row continues with y0+1, so a 512B gather at row r delivers
[a, c, b, d] = the four bilinear corners (x/y clipping is baked in by
edge-padding the table).

Self-contained: hardcodes shapes; host prep is data-independent only.
"""

import sys

for _p in ("/opt/trn_rl_repo", "/root/.axon_site/_ro/trn_rl_repo"):
    if _p not in sys.path:
        sys.path.insert(0, _p)

import numpy as np
import ml_dtypes

import concourse.bass as bass
import concourse.mybir as mybir
import concourse.tile as tile
from concourse import bacc
from concourse.masks import make_identity

F32 = mybir.dt.float32
BF16 = mybir.dt.bfloat16
I16 = mybir.dt.int16
OP = mybir.AluOpType
AF = mybir.ActivationFunctionType

# problem constants
B, H, W, C = 4, 128, 128, 128
G = 2
Cg = C // G          # 64
K2 = 9
Kin = K2 * Cg        # 576
Fg = 64
YP = H + 1           # y-entries per x' column in the gather table (129)
NROW = W * YP        # 16512 table rows
NPIX = H * W
STR = 16             # output rows per stripe
NSTRIPE = H // STR   # 8
WR = STR + 2         # sampled-row window per stripe (halo)
NI = WR * 128        # idxs per gather instr = 2304
SLOTS_I = NI // 16   # 144
SLOTS_T = NSTRIPE * K2 * SLOTS_I   # 10368
NCHUNK = 5           # 576 -> 5 chunks of 128 (last zero-padded)
NF = K2 * H          # 1152


def _build_program(debug=False):
    nc = bacc.Bacc("TRN2", target_bir_lowering=False, debug=False,
                   enable_asserts=False)
    dbg = {}
    with tile.TileContext(nc) as tc:
        with tc.tile_pool(name="dram", bufs=1, space="DRAM") as dram:
            xT_d = dram.tile([Cg, H + 2, W + 2], F32, kind="ExternalInput")
            xg_d = dram.tile([NROW, 4 * Cg], BF16, kind="ExternalInput")
            b0_d = dram.tile([2 * K2, NPIX], F32, kind="ExternalInput")
            offw_d = dram.tile([Cg, K2 * 2 * K2], F32, kind="ExternalInput")
            wd_d = dram.tile([128, K2 * NCHUNK * Fg], BF16, kind="ExternalInput")
            bfin_d = dram.tile([Fg, 1], F32, kind="ExternalInput")
            out_d = dram.tile([Fg, NPIX], F32, kind="ExternalOutput")
            locx_d = dram.tile([K2 * NPIX], F32)   # [k][px][py]
            locy_d = dram.tile([K2 * NPIX], F32)
            idxT_d = dram.tile([K2, H, 128], I16)  # [k][py][px]
            if debug:
                dbg["loc"] = dram.tile([2 * K2, NPIX], F32, kind="ExternalOutput",
                                       name="dbg_loc")
                dbg["samp"] = dram.tile([128, NSTRIPE * WR * K2 * Cg], BF16,
                                        kind="ExternalOutput", name="dbg_samp")
                dbg["idx"] = dram.tile([128, NF], F32, kind="ExternalOutput",
                                       name="dbg_idx")
                dbg["wab"] = dram.tile([128, 1154], BF16, kind="ExternalOutput",
                                       name="dbg_wab")
                dbg["idxw"] = dram.tile([128, SLOTS_T], I16,
                                        kind="ExternalOutput", name="dbg_idxw")

            with tc.tile_pool(name="persist", bufs=1) as pp, \
                 tc.tile_pool(name="pidx", bufs=1) as pidx:
                idxw = pidx.tile([128, SLOTS_T], I16)
                nc.vector.memset(idxw[:], 0)
                wab = pp.tile([128, 1154], BF16)
                wbb = pp.tile([128, 1154], BF16)
                wcb = pp.tile([128, 1154], BF16)
                wdb = pp.tile([128, 1154], BF16)
                wd_sb = pp.tile([128, K2 * NCHUNK * Fg], BF16)
                bfin = pp.tile([Fg, 1], F32)
                identb = pp.tile([128, 128], BF16)
                identf = pp.tile([128, 128], F32)

                nc.sync.dma_start(wd_sb[:], wd_d[:])
                nc.sync.dma_start(bfin[:], bfin_d[:])
                make_identity(nc, identb[:])
                make_identity(nc, identf[:])
                for wt in (wab, wbb, wcb, wdb):
                    nc.vector.memset(wt[:, 0:1], 0.0)
                    nc.vector.memset(wt[:, 1153:1154], 0.0)

                # ---- phase 1: offset conv; locA free dim is px-major ----
                # ---- phase 2: bilinear math in pg-permuted partitions ----
                with tc.tile_pool(name="ph1", bufs=1) as p1x:
                    locA = p1x.tile([2 * K2, NPIX], F32)
                    with tc.tile_pool(name="ph1a", bufs=1) as p1a, \
                         tc.tile_pool(name="ph1b", bufs=2) as p1b, \
                         tc.tile_pool(name="ps1", bufs=2, space="PSUM") as ps1:
                      xT = p1a.tile([Cg, H + 2, W + 2], F32)
                      offw = p1a.tile([Cg, K2 * 2 * K2], F32)
                      nc.sync.dma_start(xT[:], xT_d[:])
                      nc.sync.dma_start(offw[:], offw_d[:])
                      for t in range(NPIX // 512):   # 4 px-columns per tile
                        c0 = t * 4
                        b0t = p1b.tile([2 * K2, 512], F32, tag="b0t")
                        nc.sync.dma_start(b0t[:], b0_d[:, t * 512:(t + 1) * 512])
                        pt = ps1.tile([2 * K2, 512], F32, space="PSUM")
                        for tap in range(K2):
                            dy, dx = tap // 3, tap % 3
                            rhs = xT[:, dy:dy + H,
                                     c0 + dx:c0 + dx + 4].rearrange(
                                         "c y x -> c x y")
                            nc.tensor.matmul(
                                out=pt[:],
                                lhsT=offw[:, tap * 18:(tap + 1) * 18],
                                rhs=rhs,
                                start=(tap == 0), stop=(tap == K2 - 1))
                        nc.vector.tensor_tensor(
                            out=locA[:, t * 512:(t + 1) * 512],
                            in0=pt[:], in1=b0t[:], op=OP.add)
                    if debug:
                        nc.sync.dma_start(dbg["loc"][:], locA[:])
                    # bounce locA through DRAM ([k][px][py])
                    nc.sync.dma_start(
                        locx_d[:].rearrange("(a b) -> a b", a=K2), locA[0:K2, :])
                    nc.sync.dma_start(
                        locy_d[:].rearrange("(a b) -> a b", a=K2),
                        locA[K2:2 * K2, :])

                with tc.tile_pool(name="ph2", bufs=1) as p1:
                    # phase 2 tiles (pg-partition order, free = (k, py))
                    locx = p1.tile([128, NF], F32)
                    locy = p1.tile([128, NF], F32)
                    nc.vector.memset(locx[:], 0.0)
                    nc.vector.memset(locy[:], 0.0)
                    # load in pg-partition order: partitions pg=16u+b,
                    # free (k, py); src px = 8b+u.
                    for (dst, src_d) in ((locx, locx_d), (locy, locy_d)):
                        sv = src_d[:].rearrange("(k x y) -> x k y", k=K2, x=W)
                        for u in range(8):
                            sap = sv[u::8]       # px = 8b+u, b=0..15
                            dd = dst[16 * u:16 * (u + 1), :].rearrange(
                                "p (k y) -> p k y", k=K2)
                            nc.sync.dma_start(dd, sap)

                    fr = p1.tile([128, NF], F32)
                    x0f = p1.tile([128, NF], F32)
                    y0f = p1.tile([128, NF], F32)
                    c1f = p1.tile([128, NF], F32)
                    wx0f = p1.tile([128, NF], F32)
                    wx1f = p1.tile([128, NF], F32)
                    wy0f = p1.tile([128, NF], F32)
                    wy1f = p1.tile([128, NF], F32)

                    for loc, c0f, w0, w1 in (
                            (locx, x0f, wx0f, wx1f),
                            (locy, y0f, wy0f, wy1f)):
                        nc.vector.tensor_scalar(out=loc[:], in0=loc[:],
                                                scalar1=0.0, scalar2=float(W - 1),
                                                op0=OP.max, op1=OP.min)
                        # exact floor: r = round(loc) via 2^23 trick,
                        # then subtract 1 where r > loc
                        nc.vector.tensor_scalar(out=c0f[:], in0=loc[:],
                                                scalar1=8388608.0,
                                                scalar2=-8388608.0,
                                                op0=OP.add, op1=OP.add)
                        nc.vector.tensor_tensor(out=fr[:], in0=c0f[:],
                                                in1=loc[:], op=OP.is_gt)
                        nc.vector.tensor_sub(out=c0f[:], in0=c0f[:], in1=fr[:])
                        nc.vector.tensor_scalar(out=c1f[:], in0=c0f[:],
                                                scalar1=1.0, scalar2=float(W - 1),
                                                op0=OP.add, op1=OP.min)
                        nc.vector.tensor_sub(out=w0[:], in0=c1f[:], in1=loc[:])
                        nc.vector.tensor_sub(out=w1[:], in0=loc[:], in1=c0f[:])

                    # combined gather-row index: idx = 129*x0 + y0
                    nc.vector.scalar_tensor_tensor(
                        out=fr[:], in0=x0f[:], scalar=float(YP), in1=y0f[:],
                        op0=OP.mult, op1=OP.add)
                    # pre-multiplied corner weights (bf16)
                    nc.vector.tensor_tensor(out=wab[:, 1:1153], in0=wx0f[:],
                                            in1=wy0f[:], op=OP.mult)
                    nc.vector.tensor_tensor(out=wbb[:, 1:1153], in0=wx0f[:],
                                            in1=wy1f[:], op=OP.mult)
                    nc.vector.tensor_tensor(out=wcb[:, 1:1153], in0=wx1f[:],
                                            in1=wy0f[:], op=OP.mult)
                    nc.vector.tensor_tensor(out=wdb[:, 1:1153], in0=wx1f[:],
                                            in1=wy1f[:], op=OP.mult)

                    # transpose each k-block to [py, px-natural] int16, then
                    # scatter into wrapped idx layout.
                    with tc.tile_pool(name="tpi", bufs=1) as tpi, \
                         tc.tile_pool(name="pst2", bufs=4, space="PSUM") as pst2:
                        tT = [tpi.tile([128, 128], I16, name=f"tT_{k}")
                              for k in range(K2)]
                        for k in range(K2):
                            ptr = pst2.tile([128, 128], F32, space="PSUM",
                                            tag="ptr")
                            nc.tensor.transpose(
                                out=ptr[:],
                                in_=fr[:, k * H:(k + 1) * H],
                                identity=identf[:])
                            # un-permute pg -> px while casting to int16
                            src = ptr[:].rearrange("p (u b) -> p u b", u=8)
                            dd = tT[k][:].rearrange("p (b u) -> p u b", b=16)
                            nc.scalar.copy(out=dd, in_=src)
                        # bounce tT through DRAM [k][py][px], then
                        # scatter per (s, k) into the wrapped layout
                        for k in range(K2):
                            nc.sync.dma_start(idxT_d[k, :, :], tT[k][:])
                        for k in range(K2):
                            def sc(s, w_lo, w_hi, py0, k=k):
                                cnt = w_hi - w_lo
                                src = idxT_d[k, py0:py0 + cnt, :].rearrange(
                                    "w (b u) -> b w u", b=16)
                                o0 = (s * K2 + k) * SLOTS_I + 8 * w_lo
                                dd = idxw[0:16, o0:o0 + cnt * 8].rearrange(
                                    "p (w u) -> p w u", u=8)
                                nc.sync.dma_start(dd, src)

                            for s in range(NSTRIPE):
                                if s == 0:
                                    sc(s, 0, 1, 0)
                                    sc(s, 1, WR, 0)
                                elif s == NSTRIPE - 1:
                                    sc(s, 0, WR - 1, STR * s - 1)
                                    sc(s, WR - 1, WR, H - 1)
                                else:
                                    sc(s, 0, WR, STR * s - 1)
                        for a in range(1, 8):
                            nc.sync.dma_start(idxw[16 * a:16 * (a + 1), :],
                                              idxw[0:16, :])
                        if debug:
                            nc.sync.dma_start(dbg["idx"][:], fr[:])
                            nc.sync.dma_start(dbg["wab"][:], wab[:])
                            nc.sync.dma_start(dbg["idxw"][:], idxw[:])

                # ---- phase 3: gather / combine / transpose / dwpw ----
                with tc.tile_pool(name="gb", bufs=2) as gbp, \
                     tc.tile_pool(name="cmb", bufs=2) as cmb, \
                     tc.tile_pool(name="samp", bufs=1) as smp, \
                     tc.tile_pool(name="outp", bufs=2) as outp, \
                     tc.tile_pool(name="pst", bufs=4, space="PSUM") as pst, \
                     tc.tile_pool(name="psm", bufs=2, space="PSUM") as psm:
                    samp = smp.tile([128, WR, K2, Cg], BF16)
                    sampT = [smp.tile([128, WR, W + 2], BF16, name=f"sampT{i}")
                             for i in range(NCHUNK)]
                    for i in range(NCHUNK):
                        nc.vector.memset(sampT[i][:, :, 0:1], 0.0)
                        nc.vector.memset(sampT[i][:, :, W + 1:W + 2], 0.0)
                    nc.vector.memset(sampT[4][64:128, :, :], 0.0)

                    import os as _os
                    _ns = int(_os.environ.get("KSTRIPES", NSTRIPE))
                    for s in range(_ns):
                        for k in range(K2):
                            gbt = gbp.tile([128, WR, 4 * Cg], BF16, tag="gt")
                            off = (s * K2 + k) * SLOTS_I
                            for c3 in range(3):   # 6 w-rows per gather
                                nc.gpsimd.dma_gather(
                                    out_ap=gbt[:, 6 * c3:6 * (c3 + 1), :],
                                    in_ap=xg_d[:],
                                    idxs_ap=idxw[:, off + 48 * c3:
                                                  off + 48 * (c3 + 1)],
                                    num_idxs=768, num_idxs_reg=768,
                                    elem_size=4 * Cg)
                            if int(_os.environ.get("KPARTS", 4)) < 2:
                                continue
                            a_ = gbt[:, :, 0:Cg]
                            c_ = gbt[:, :, Cg:2 * Cg]
                            b_ = gbt[:, :, 2 * Cg:3 * Cg]
                            d_ = gbt[:, :, 3 * Cg:4 * Cg]
                            wsl = slice(k * H + STR * s, k * H + STR * s + WR)
                            wa = wab[:, wsl].to_broadcast([128, WR, Cg])
                            wb = wbb[:, wsl].to_broadcast([128, WR, Cg])
                            wc = wcb[:, wsl].to_broadcast([128, WR, Cg])
                            wd = wdb[:, wsl].to_broadcast([128, WR, Cg])
                            t1 = cmb.tile([128, WR, Cg], BF16, tag="t1")
                            t2 = cmb.tile([128, WR, Cg], BF16, tag="t2")
                            t3 = cmb.tile([128, WR, Cg], BF16, tag="t3")
                            nc.vector.tensor_tensor(out=t1[:], in0=a_, in1=wa, op=OP.mult)
                            nc.vector.tensor_tensor(out=t2[:], in0=c_, in1=wc, op=OP.mult)
                            nc.vector.tensor_tensor(out=t1[:], in0=t1[:], in1=t2[:], op=OP.add)
                            nc.vector.tensor_tensor(out=t2[:], in0=b_, in1=wb, op=OP.mult)
                            nc.vector.tensor_tensor(out=t3[:], in0=d_, in1=wd, op=OP.mult)
                            nc.vector.tensor_tensor(out=t2[:], in0=t2[:], in1=t3[:], op=OP.add)
                            nc.vector.tensor_tensor(
                                out=samp[:, :, k, :], in0=t1[:], in1=t2[:], op=OP.add)
                        if debug:
                            nc.sync.dma_start(
                                dbg["samp"][:, s * WR * Kin:(s + 1) * WR * Kin],
                                samp[:].rearrange("p a b c -> p (a b c)"))
                        if int(_os.environ.get("KPARTS", 4)) < 3:
                            continue
                        # transposes into sampT (un-permuting pg -> px)
                        w_lo = 1 if s == 0 else 0
                        w_hi = WR - 1 if s == NSTRIPE - 1 else WR
                        if s == 0:
                            for i in range(NCHUNK):
                                nc.vector.memset(sampT[i][:, 0, :], 0.0)
                        if s == NSTRIPE - 1:
                            for i in range(NCHUNK):
                                nc.vector.memset(sampT[i][:, WR - 1, :], 0.0)
                        for wrow in range(w_lo, w_hi):
                            for kp in range(NCHUNK):
                                kk = 2 * kp
                                width = 128 if kp < 4 else 64
                                src = samp[:, wrow, kk:kk + (2 if kp < 4 else 1), :]
                                ptt = pst.tile([128, 128], BF16, space="PSUM",
                                               tag="ptt")
                                nc.tensor.transpose(
                                    out=ptt[:width, :],
                                    in_=src.rearrange("p a b -> p (a b)"),
                                    identity=identb[:])
                                src2 = ptt[:width, :].rearrange(
                                    "p (u b) -> p u b", u=8)
                                dd = sampT[kp][:width, wrow, 1:1 + W].rearrange(
                                    "p (b u) -> p u b", b=16)
                                nc.scalar.copy(out=dd, in_=src2)
                        if int(_os.environ.get("KPARTS", 4)) < 4:
                            continue
                        # dwpw matmuls
                        for t in range(4):
                            pm = psm.tile([Fg, 512], F32, space="PSUM", tag="pm")
                            first = True
                            for dy in (-1, 0, 1):
                                for dx in (-1, 0, 1):
                                    d_i = (dy + 1) * 3 + (dx + 1)
                                    for ci in range(NCHUNK):
                                        lhs = wd_sb[:, (d_i * NCHUNK + ci) * Fg:
                                                    (d_i * NCHUNK + ci + 1) * Fg]
                                        wr0 = t * 4 + 1 + dy
                                        rhs = sampT[ci][:, wr0:wr0 + 4,
                                                        1 + dx:1 + dx + W]
                                        last = (dy == 1 and dx == 1 and
                                                ci == NCHUNK - 1)
                                        nc.tensor.matmul(out=pm[:], lhsT=lhs,
                                                         rhs=rhs, start=first,
                                                         stop=last)
                                        first = False
                            ot = outp.tile([Fg, 512], F32, tag="ot")
                            nc.scalar.activation(out=ot[:], in_=pm[:],
                                                 func=AF.Identity, bias=bfin[:],
                                                 scale=1.0)
                            nc.sync.dma_start(
                                out_d[:, s * 2048 + t * 512:
                                      s * 2048 + (t + 1) * 512],
                                ot[:])
    nc.compile()
    names = dict(xT=xT_d.name, xg=xg_d.name, b0=b0_d.name, offw=offw_d.name,
                 wd=wd_d.name, bfin=bfin_d.name, out=out_d.name,
                 dbg={k: v.name for k, v in dbg.items()})
    return nc, names


def _host_prep(x, off_w, off_b, dw_w, dw_b, pw_w, pw_b, b, g):
    """Data-independent prep of one (b, g) shard's device inputs."""
    xi = np.asarray(x)[b, :, :, g * Cg:(g + 1) * Cg].astype(np.float32)
    xT = np.zeros((Cg, H + 2, W + 2), np.float32)
    xT[:, 1:H + 1, 1:W + 1] = xi.transpose(2, 0, 1)
    # gather table: row r = x0*129 + y0 -> [X[y0,x0], X[y0,x0+1]] (2*Cg);
    # 512B gathers read rows (r, r+1) = the 2x2 corner block [a, c, b, d].
    xgl = np.pad(xi, ((0, 1), (0, 1), (0, 0)), mode="edge")  # [129,129,Cg]
    pair = np.concatenate([xgl[:, :W], xgl[:, 1:W + 1]], axis=2)  # [129,128,2Cg]
    t1 = np.ascontiguousarray(pair.transpose(1, 0, 2)).reshape(NROW, 2 * Cg)
    t1p = np.concatenate([t1, t1[-1:]], axis=0)              # [16513, 2Cg]
    xg = np.concatenate([t1p[:-1], t1p[1:]], axis=1)         # [16512, 4Cg]
    xg = xg.astype(ml_dtypes.bfloat16)
    # base tables [18, NPIX], free dim px-major (px*H + py)
    lin = np.array([-1.0, 0.0, 1.0], np.float32)
    gx, gy = np.meshgrid(np.arange(W, dtype=np.float32),
                         np.arange(H, dtype=np.float32))
    gxT, gyT = gx.T.reshape(-1), gy.T.reshape(-1)   # px-major flatten
    ob = np.asarray(off_b)[g].astype(np.float32)
    b0 = np.zeros((2 * K2, NPIX), np.float32)
    for k in range(K2):
        b0[k] = gxT + lin[k % 3] + ob[2 * k]
        b0[K2 + k] = gyT + lin[k // 3] + ob[2 * k + 1]
    ow = np.asarray(off_w)[g].astype(np.float32)
    offw = np.zeros((Cg, K2 * 2 * K2), np.float32)
    for tap in range(K2):
        wt = ow[tap // 3, tap % 3]
        offw[:, tap * 18:tap * 18 + K2] = wt[:, 0::2]
        offw[:, tap * 18 + K2:(tap + 1) * 18] = wt[:, 1::2]
    dw = np.asarray(dw_w)[g, :, :, 0, :].astype(np.float32)
    pw = np.asarray(pw_w)[g, 0, 0].astype(np.float32)
    wd = np.zeros((128, K2 * NCHUNK * Fg), np.float32)
    for d_i in range(K2):
        wfull = dw[d_i // 3, d_i % 3][:, None] * pw
        for ci in range(NCHUNK):
            rows = min(128, Kin - ci * 128)
            wd[:rows, (d_i * NCHUNK + ci) * Fg:(d_i * NCHUNK + ci + 1) * Fg] = \
                wfull[ci * 128:ci * 128 + rows]
    wd = wd.astype(ml_dtypes.bfloat16)
    bfin = (pw.T @ np.asarray(dw_b)[g].astype(np.float32)
            + np.asarray(pw_b)[g].astype(np.float32)).reshape(Fg, 1)
    return dict(xT=xT, xg=xg, b0=b0, offw=offw, wd=wd, bfin=bfin)


_CACHE = {}


def _get_program(debug=False):
    key = ("prog", debug)
    if key not in _CACHE:
        _CACHE[key] = _build_program(debug=debug)
    return _CACHE[key]


LAST_RESULT = None


def kernel(x, off_w, off_b, dw_w, dw_b, pw_w, pw_b):
    import os
    from concourse import bass_utils
    nc, names = _get_program()
    shards = [(b, g) for b in range(B) for g in range(G)]
    in_maps = []
    for b, g in shards:
        prep = _host_prep(x, off_w, off_b, dw_w, dw_b, pw_w, pw_b, b, g)
        in_maps.append({names[k]: v for k, v in prep.items()})
    kw = {}
    if os.environ.get("KERNEL_TRACE") == "1":
        kw = dict(trace=True)
        td = os.environ.get("KERNEL_TRACE_DIR")
        if td:
            kw["tmpdir"] = td
    res = bass_utils.run_bass_kernel_spmd(nc, in_maps, core_ids=list(range(8)),
                                          **kw)
    global LAST_RESULT
    LAST_RESULT = res
    out = np.zeros((B, H, W, C), np.float32)
    for i, (b, g) in enumerate(shards):
        o = np.asarray(res.results[i][names["out"]])  # [Fg, NPIX]
        out[b, :, :, g * Cg:(g + 1) * Cg] = \
            o.reshape(Fg, H, W).transpose(1, 2, 0)
    return out
